# revision 1
# baseline (speedup 1.0000x reference)
"""2-layer GCN (DGL GraphConv, norm='both') on 8 Trainium2 cores.

Math restructure: the symmetric-normalized aggregation A_hat = D_dst^-1/2 A^T
D_src^-1/2 is linear over the feature axis, so it commutes with the weight
matmul: (A_hat X) W == A_hat (X W). We therefore run the irregular sparse
aggregation on host (sorted segment reduce) and ship only the dense GEMMs
(g @ W per node shard) to the NeuronCores: nodes are range-sharded 8 ways,
weights replicated, activations shipped transposed [128, nodes] so the
stationary operand of the tensor engine is the 128x128 weight.
"""

import sys

sys.path.insert(0, "/opt/trn_rl_repo")

import numpy as np

N = 100000
D = 128
NCORES = 8
SH = N // NCORES          # 12500 nodes per core
CH = 512                  # psum free-dim chunk (one fp32 bank)
SHP = 13312               # 26 * 512, padded shard width
NCHUNK = SHP // CH
NPSUM = 4

_NC_CACHE = {}


def _build_nc():
    import concourse.bass as bass
    import concourse.mybir as mybir

    f32 = mybir.dt.float32
    AP = bass.AP
    nc = bass.Bass()

    gT = nc.dram_tensor("gT", [D, SHP], f32, kind="ExternalInput")
    W = nc.dram_tensor("W", [D, D], f32, kind="ExternalInput")
    oT = nc.dram_tensor("oT", [D, SHP], f32, kind="ExternalOutput")

    ctx_tensors = []
    with (
        nc.semaphore("dma_sem") as dma_sem,
        nc.semaphore("mm_sem") as mm_sem,
        nc.semaphore("cp_sem") as cp_sem,
        nc.sbuf_tensor("g_sb", [D, SHP], f32) as g_sb,
        nc.sbuf_tensor("w_sb", [D, D], f32) as w_sb,
        nc.sbuf_tensor("o_sb", [D, SHP], f32) as o_sb,
        nc.psum_tensor("ps0", [D, CH], f32) as ps0,
        nc.psum_tensor("ps1", [D, CH], f32) as ps1,
        nc.psum_tensor("ps2", [D, CH], f32) as ps2,
        nc.psum_tensor("ps3", [D, CH], f32) as ps3,
    ):
        ps = [ps0, ps1, ps2, ps3]

        def sb_chunk(t, j):
            return AP(t, j * CH, [[SHP, D], [1, CH]])

        def ps_chunk(p):
            return AP(p, 0, [[CH, D], [1, CH]])

        with nc.Block() as block:

            @block.sync
            def _(sync):
                sync.dma_start(
                    AP(w_sb, 0, [[D, D], [1, D]]),
                    AP(W, 0, [[D, D], [1, D]]),
                ).then_inc(dma_sem, 16)
                for j in range(NCHUNK):
                    sync.dma_start(
                        sb_chunk(g_sb, j),
                        AP(gT, j * CH, [[SHP, D], [1, CH]]),
                    ).then_inc(dma_sem, 16)

            @block.tensor
            def _(tensor):
                tensor.wait_ge(dma_sem, 16 * (NCHUNK + 1))
                for j in range(NCHUNK):
                    if j >= 2:
                        tensor.wait_ge(cp_sem, j - 1)
                    tensor.matmul(
                        ps_chunk(ps[j % NPSUM]),
                        AP(w_sb, 0, [[D, D], [1, D]]),
                        sb_chunk(g_sb, j),
                        start=True,
                        stop=True,
                    ).then_inc(mm_sem)

            @block.vector
            def _(vector):
                for j in range(NCHUNK):
                    vector.wait_ge(mm_sem, j + 1)
                    vector.tensor_scalar_add(
                        sb_chunk(o_sb, j), ps_chunk(ps[j % NPSUM]), 0.0
                    ).then_inc(cp_sem)

            @block.gpsimd
            def _(gpsimd):
                for j in range(NCHUNK):
                    gpsimd.wait_ge(cp_sem, j + 1)
                    gpsimd.dma_start(
                        AP(oT, j * CH, [[SHP, D], [1, CH]]),
                        sb_chunk(o_sb, j),
                    ).then_inc(dma_sem, 16)
                gpsimd.wait_ge(dma_sem, 16 * (2 * NCHUNK + 1))

    del ctx_tensors
    return nc


def _get_nc():
    if "nc" not in _NC_CACHE:
        _NC_CACHE["nc"] = _build_nc()
    return _NC_CACHE["nc"]


def _device_gemm(g_full, Wm):
    """g_full [N,128] @ Wm [128,128] across 8 cores; returns [N,128]."""
    from concourse.bass_utils import run_bass_kernel_spmd

    nc = _get_nc()
    Wc = np.ascontiguousarray(Wm, dtype=np.float32)
    in_maps = []
    for i in range(NCORES):
        shard = g_full[i * SH : (i + 1) * SH]  # [SH, D]
        gT = np.zeros((D, SHP), dtype=np.float32)
        gT[:, :SH] = shard.T
        in_maps.append({"gT": gT, "W": Wc})
    res = run_bass_kernel_spmd(nc, in_maps, list(range(NCORES)))
    outs = [res.results[i]["oT"][:, :SH].T for i in range(NCORES)]
    return np.concatenate(outs, axis=0)


def kernel(feat, src, dst, W1, b1, W2, b2):
    feat = np.asarray(feat, dtype=np.float32)
    src = np.asarray(src, dtype=np.int64)
    dst = np.asarray(dst, dtype=np.int64)
    W1 = np.asarray(W1, dtype=np.float32)
    b1 = np.asarray(b1, dtype=np.float32)
    W2 = np.asarray(W2, dtype=np.float32)
    b2 = np.asarray(b2, dtype=np.float32)

    out_deg = np.bincount(src, minlength=N).astype(np.float32)
    in_deg = np.bincount(dst, minlength=N).astype(np.float32)
    ns = 1.0 / np.sqrt(np.maximum(out_deg, 1.0))
    nd = 1.0 / np.sqrt(np.maximum(in_deg, 1.0))

    order = np.argsort(dst, kind="stable")
    ds_sorted = dst[order]
    starts = np.flatnonzero(
        np.concatenate(([True], ds_sorted[1:] != ds_sorted[:-1]))
    )
    seg_ids = ds_sorted[starts]
    gsrc = src[order]

    def aggregate(x):
        x1 = x * ns[:, None]
        contrib = x1[gsrc]
        sums = np.add.reduceat(contrib, starts, axis=0)
        g = np.zeros((N, D), dtype=np.float32)
        g[seg_ids] = sums
        g *= nd[:, None]
        return g

    g1 = aggregate(feat)
    h1 = _device_gemm(g1, W1) + b1
    np.maximum(h1, 0.0, out=h1)
    g2 = aggregate(h1)
    out = _device_gemm(g2, W2) + b2
    return out.astype(np.float32)



# revision 11
# speedup vs baseline: 771.4379x; 771.4379x over previous
"""2-layer GCN (DGL GraphConv, norm='both') fully on 8 Trainium2 cores.

Strategy (graph-parallel, per the sharding hint):
  - Nodes range-sharded 8 ways by dst; edges live with their dst core.
  - Per core, dst nodes tile in groups of 128. Edges are bucketed per
    (tile, src-quartile) into fixed-capacity slots (capacity = max over
    cores, 128-aligned) so all 8 cores share ONE program (SPMD).
  - Messages x[src] are fetched with the hardware dma_gather (int16 row
    indices into 4 sub-tables of 25600 rows; pad lanes gather row 0).
  - Aggregation is matmul-based: per 128-edge block a one-hot matrix
    E[j,i] = (dst_j == tile_i) * ns[src_j] is built by one DVE
    tensor_scalar(is_equal, mult) from an iota row and per-lane scalars;
    PE accumulates E^T @ M into PSUM per dst tile -> g = sum ns*x[src].
  - Per tile: g *= nd (per-partition), PE-transpose, W GEMM, (+bias),
    ReLU -> bf16 layer-2 gather table; AllGather shares the table across
    cores between layers; layer 2 repeats and writes fp32 output shards.
  - Weights replicated; degrees/norms + edge bucketing on host (index
    preprocessing); all O(E*D) / O(N*D^2) float work on device.
"""

import sys
import time

sys.path.insert(0, "/opt/trn_rl_repo")

import numpy as np

N_NODES = 100000
D = 128
NCORES = 8
STAB = 25600          # sub-table rows (int16-index addressable)
RG = 7                # tiles per gather range
GCAP = 1024           # max lanes per dma_gather instruction
LAST_EXEC_NS = None

_CACHE = {}


def _cdiv(a, b):
    return (a + b - 1) // b


def _ceil128(x):
    return (int(x) + 127) // 128 * 128


# --------------------------------------------------------------------------
# Host preprocessing: degrees, norms, edge bucketing into SPMD-uniform slots
# --------------------------------------------------------------------------

def _preprocess(src, dst, N, ncores, stab, rg):
    SH = N // ncores
    T = _cdiv(SH, 128)
    Q = _cdiv(N, stab)
    tsize = [min(128, SH - t * 128) for t in range(T)]

    out_deg = np.bincount(src, minlength=N).astype(np.float32)
    in_deg = np.bincount(dst, minlength=N).astype(np.float32)
    ns = 1.0 / np.sqrt(np.maximum(out_deg, 1.0))
    nd = 1.0 / np.sqrt(np.maximum(in_deg, 1.0))

    c = dst // SH
    t = (dst % SH) // 128
    q = src // stab
    key = (c * T + t) * Q + q
    order = np.argsort(key, kind="stable")
    skey = key[order]
    ssrc = src[order]
    sdst = dst[order]

    counts = np.bincount(key, minlength=ncores * T * Q).reshape(ncores, T, Q)
    CAP = np.zeros((T, Q), dtype=np.int64)
    for ti in range(T):
        for qi in range(Q):
            CAP[ti, qi] = _ceil128(counts[:, ti, qi].max())

    offs = np.zeros((Q, T + 1), dtype=np.int64)
    for qi in range(Q):
        offs[qi, 1:] = np.cumsum(CAP[:, qi])
    Lq = [int(offs[qi, T]) for qi in range(Q)]

    # per-edge lane position within its (c,) stream-q layout
    gstart = np.zeros(ncores * T * Q + 1, dtype=np.int64)
    gstart[1:] = np.cumsum(counts.reshape(-1))
    rank = np.arange(len(order), dtype=np.int64) - gstart[skey]
    sq = ssrc // stab
    st = (sdst % SH) // 128
    lane = offs[sq, st] + rank  # position within stream q of its core

    # per-core, per-q arrays
    idx_sb = [[None] * Q for _ in range(ncores)]
    dstoff_sb = [[None] * Q for _ in range(ncores)]
    nsrc_sb = [[None] * Q for _ in range(ncores)]
    sc = sdst // SH
    for ci in range(ncores):
        cm = sc == ci
        for qi in range(Q):
            m = cm & (sq == qi)
            L = Lq[qi]
            idx = np.zeros(L, dtype=np.int16)
            dof = np.full(L, -1.0, dtype=np.float32)
            nsr = np.zeros(L, dtype=np.float32)
            ln = lane[m]
            idx[ln] = (ssrc[m] - qi * stab).astype(np.int16)
            dof[ln] = (sdst[m] - (ci * SH + st[m] * 128)).astype(np.float32)
            nsr[ln] = ns[ssrc[m]]
            # SBUF layouts
            i16 = idx.reshape(L // 16, 16).T  # [16, L/16]
            idx_sb[ci][qi] = np.tile(i16, (8, 1)).copy()  # [128, L/16]
            dstoff_sb[ci][qi] = dof.reshape(L // 128, 128).T.copy()  # [128, L/128]
            nsrc_sb[ci][qi] = nsr.reshape(L // 128, 128).T.copy()

    ndcol = []
    for ci in range(ncores):
        v = np.ones((128, T), dtype=np.float32)
        for ti in range(T):
            ts = tsize[ti]
            v[:ts, ti] = nd[ci * SH + ti * 128: ci * SH + ti * 128 + ts]
        ndcol.append(v)

    ranges = [(t0, min(rg, T - t0)) for t0 in range(0, T, rg)]

    return dict(
        SH=SH, T=T, Q=Q, tsize=tsize, CAP=CAP, offs=offs, Lq=Lq,
        ranges=ranges, idx_sb=idx_sb, dstoff_sb=dstoff_sb, nsrc_sb=nsrc_sb,
        ndcol=ndcol, ns=ns, nd=nd,
    )


# --------------------------------------------------------------------------
# Bass program
# --------------------------------------------------------------------------

def _build_nc(S, N, stab, has_b1, has_b2):
    import concourse.bacc as bacc
    import concourse.bass as bass
    import concourse.mybir as mybir
    from concourse.tile import TileContext

    f32 = mybir.dt.float32
    bf16 = mybir.dt.bfloat16
    i16 = mybir.dt.int16
    AP = bass.AP
    AF = mybir.ActivationFunctionType
    ALU = mybir.AluOpType

    SH, T, Q = S["SH"], S["T"], S["Q"]
    CAP, offs, Lq, ranges, tsize = S["CAP"], S["offs"], S["Lq"], S["ranges"], S["tsize"]

    nc = bacc.Bacc("TRN2")

    table1 = nc.dram_tensor("table1", [N, D], bf16, kind="ExternalInput")
    idx_t = [nc.dram_tensor(f"idx{q}", [128, Lq[q] // 16], i16, kind="ExternalInput")
             for q in range(Q)]
    dof_t = [nc.dram_tensor(f"dof{q}", [128, Lq[q] // 128], f32, kind="ExternalInput")
             for q in range(Q)]
    nsr_t = [nc.dram_tensor(f"nsr{q}", [128, Lq[q] // 128], f32, kind="ExternalInput")
             for q in range(Q)]
    ndcol_t = nc.dram_tensor("ndcol", [128, T], f32, kind="ExternalInput")
    r128_t = nc.dram_tensor("r128", [128, 128], bf16, kind="ExternalInput")
    id128_t = nc.dram_tensor("id128", [128, 128], f32, kind="ExternalInput")
    w_t = [nc.dram_tensor(f"w{li}", [D, D], f32, kind="ExternalInput")
           for li in range(2)]
    b_t = [None, None]
    if has_b1:
        b_t[0] = nc.dram_tensor("b0", [1, D], f32, kind="ExternalInput")
    if has_b2:
        b_t[1] = nc.dram_tensor("b1", [1, D], f32, kind="ExternalInput")

    table2own = nc.dram_tensor("table2own", [SH, D], bf16, kind="Internal")
    table2 = nc.dram_tensor("table2", [N, D], bf16, kind="Internal")
    out_t = nc.dram_tensor("out", [SH, D], f32, kind="ExternalOutput")

    # precomputed block lists: per tile -> [(q, gcol)] ; gcol = stream block col
    tile_blocks = []
    for ti in range(T):
        bl = []
        for qi in range(Q):
            for bi in range(int(CAP[ti, qi]) // 128):
                bl.append((qi, int(offs[qi, ti]) // 128 + bi))
        tile_blocks.append(bl)

    with TileContext(nc) as tc:
        with (
            tc.tile_pool(name="const", bufs=1) as constp,
            tc.tile_pool(name="meta", bufs=1) as metap,
            tc.tile_pool(name="msg", bufs=2) as msgp,
            tc.tile_pool(name="oh", bufs=8) as ohp,
            tc.tile_pool(name="gsb", bufs=4) as gp,
            tc.tile_pool(name="gtsb", bufs=4) as gtp,
            tc.tile_pool(name="hsb", bufs=4) as hp,
            tc.tile_pool(name="aggps", bufs=4, space="PSUM") as aggp,
            tc.tile_pool(name="tps", bufs=2, space="PSUM") as tpp,
            tc.tile_pool(name="hps", bufs=2, space="PSUM") as hpp,
        ):
            r128 = constp.tile_from(r128_t[:, :], name="r128")
            id128 = constp.tile_from(id128_t[:, :], name="id128")
            wsb = [constp.tile_from(w_t[li][:, :], name=f"wsb{li}") for li in range(2)]
            ndc = constp.tile_from(ndcol_t[:, :], name="ndc")
            bsb = [None, None]
            ones1 = None
            if has_b1 or has_b2:
                ones1 = constp.tile([1, D], f32, tag="ones1")
                nc.vector.memset(ones1[:, :], 1.0)
                for li in range(2):
                    if b_t[li] is not None:
                        bsb[li] = constp.tile_from(b_t[li][:, :], name=f"bsb{li}")
            idxs = [metap.tile_from(idx_t[q][:, :], name=f"idxs{q}") for q in range(Q)]
            dofs = [metap.tile_from(dof_t[q][:, :], name=f"dofs{q}") for q in range(Q)]
            nsrs = [metap.tile_from(nsr_t[q][:, :], name=f"nsrs{q}") for q in range(Q)]

            for li in range(2):
                table = table1 if li == 0 else table2
                has_b = (has_b1, has_b2)[li]
                for (t0, nt) in ranges:
                    # gathers: one per stream q covering this range's slots
                    msgs = {}
                    base = {}
                    for qi in range(Q):
                        w0 = int(offs[qi, t0])
                        wlen = int(offs[qi, t0 + nt]) - w0
                        if wlen == 0:
                            continue
                        nb = wlen // 128
                        m = msgp.tile([128, nb, 128], bf16, tag=f"msg{qi}")
                        ext = min(stab, N - qi * stab)
                        in_ap = AP(table, qi * stab * D, [[D, ext], [1, D]])
                        for c0 in range(0, wlen, GCAP):
                            cl = min(GCAP, wlen - c0)
                            nc.gpsimd.dma_gather(
                                m[:, c0 // 128: (c0 + cl) // 128, :], in_ap,
                                idxs[qi][:, (w0 + c0) // 16:
                                          (w0 + c0 + cl) // 16],
                                cl, cl, D, elem_step=D,
                            )
                        msgs[qi] = m
                        base[qi] = w0 // 128

                    nbank = _cdiv(nt, 4)
                    banks = [aggp.tile([128, 512], f32, tag="aggbank",
                                       name="aggbank")
                             for _ in range(nbank)]

                    for tl in range(nt):
                        ti = t0 + tl
                        ps = banks[tl // 4][:, (tl % 4) * 128: (tl % 4) * 128 + 128]
                        bl = tile_blocks[ti]
                        for k, (qi, gcol) in enumerate(bl):
                            oh = ohp.tile([128, 128], bf16, tag="oh")
                            nc.vector.tensor_scalar(
                                oh[:, :], r128[:, :],
                                dofs[qi][:, gcol: gcol + 1],
                                nsrs[qi][:, gcol: gcol + 1],
                                ALU.is_equal, ALU.mult,
                            )
                            bcol = gcol - base[qi]
                            nc.tensor.matmul(
                                ps, oh[:, :], msgs[qi][:, bcol, :],
                                start=(k == 0), stop=(k == len(bl) - 1),
                            )
                        g = gp.tile([128, 128], f32, tag="g")
                        if bl:
                            nc.vector.tensor_scalar(
                                g[:, :], ps, ndc[:, ti: ti + 1], None, ALU.mult,
                            )
                        else:
                            nc.vector.memset(g[:, :], 0.0)
                        pt = tpp.tile([128, 128], f32, tag="tp")
                        nc.tensor.transpose(pt[:, :], g[:, :], id128[:, :])
                        gT = gtp.tile([128, 128], f32, tag="gT")
                        nc.scalar.copy(gT[:, :], pt[:, :])
                        ph = hpp.tile([128, 128], f32, tag="hp")
                        nc.tensor.matmul(ph[:, :], gT[:, :], wsb[li][:, :],
                                         start=True, stop=not has_b)
                        if has_b:
                            nc.tensor.matmul(ph[:, :], ones1[:, :], bsb[li][:, :],
                                             start=False, stop=True)
                        ts = tsize[ti]
                        if li == 0:
                            h = hp.tile([128, 128], bf16, tag="h")
                            nc.scalar.activation(h[:, :], ph[:, :], AF.Relu)
                            nc.sync.dma_start(
                                table2own[ti * 128: ti * 128 + ts, :], h[0:ts, :])
                        else:
                            o = hp.tile([128, 128], f32, tag="o")
                            nc.vector.tensor_copy(o[:, :], ph[:, :])
                            nc.sync.dma_start(
                                out_t[ti * 128: ti * 128 + ts, :], o[0:ts, :])

                if li == 0:
                    tc.strict_bb_all_engine_barrier()
                    nc.gpsimd.collective_compute(
                        "AllGather",
                        mybir.AluOpType.bypass,
                        replica_groups=[list(range(NCORES))],
                        ins=[table2own[0:SH, :]],
                        outs=[table2[0:NCORES * SH, :]],
                    )
                    tc.strict_bb_all_engine_barrier()

    nc.compile()
    return nc


# --------------------------------------------------------------------------
# PJRT execution (axon): compile once, run, and time repeat executions
# --------------------------------------------------------------------------

def _make_runner(nc, n_cores):
    import jax
    import concourse.mybir as mybir
    from jax.experimental.shard_map import shard_map
    from jax.sharding import Mesh, PartitionSpec
    from concourse.bass2jax import (
        _bass_exec_p,
        install_neuronx_cc_hook,
        partition_id_tensor,
    )

    install_neuronx_cc_hook()

    partition_name = (
        nc.partition_id_tensor.name if nc.partition_id_tensor else None
    )
    in_names, out_names, out_avals, zero_outs = [], [], [], []
    for alloc in nc.m.functions[0].allocations:
        if not isinstance(alloc, mybir.MemoryLocationSet):
            continue
        name = alloc.memorylocations[0].name
        if alloc.kind == "ExternalInput":
            if name != partition_name:
                in_names.append(name)
        elif alloc.kind == "ExternalOutput":
            shape = tuple(alloc.tensor_shape)
            dtype = mybir.dt.np(alloc.dtype)
            out_names.append(name)
            out_avals.append(jax.core.ShapedArray(shape, dtype))
            zero_outs.append(np.zeros(shape, dtype))
    n_params = len(in_names)
    all_in_names = in_names + out_names
    if partition_name is not None:
        all_in_names = all_in_names + [partition_name]
    donate = tuple(range(n_params, n_params + len(out_names)))

    def _body(*args):
        operands = list(args)
        if partition_name is not None:
            operands.append(partition_id_tensor())
        outs = _bass_exec_p.bind(
            *operands,
            out_avals=tuple(out_avals),
            in_names=tuple(all_in_names),
            out_names=tuple(out_names),
            lowering_input_output_aliases=(),
            sim_require_finite=True,
            sim_require_nnan=True,
            nc=nc,
        )
        return tuple(outs)

    devices = jax.devices()[:n_cores]
    mesh = Mesh(np.asarray(devices), ("core",))
    in_specs = (PartitionSpec("core"),) * (n_params + len(out_names))
    out_specs = (PartitionSpec("core"),) * len(out_names)
    sharded = jax.jit(
        shard_map(_body, mesh=mesh, in_specs=in_specs, out_specs=out_specs,
                  check_rep=False),
        donate_argnums=donate, keep_unused=True,
    )
    return sharded, mesh, in_names, out_names, zero_outs


def _run_and_time(nc, in_maps, n_cores, time_runs=3):
    import jax
    from jax.sharding import NamedSharding, PartitionSpec

    sharded, mesh, in_names, out_names, zero_outs = _make_runner(nc, n_cores)
    concat_in = [
        np.concatenate([np.asarray(in_maps[c][nm]) for c in range(n_cores)], axis=0)
        for nm in in_names
    ]
    concat_zeros = [
        np.zeros((n_cores * z.shape[0], *z.shape[1:]), z.dtype) for z in zero_outs
    ]
    out_arrs = sharded(*concat_in, *concat_zeros)
    results = [
        {nm: np.asarray(out_arrs[i]).reshape(n_cores, -1, *out_arrs[i].shape[1:])[c]
         for i, nm in enumerate(out_names)}
        for c in range(n_cores)
    ]

    exec_ns = None
    if time_runs > 0:
        sh = NamedSharding(mesh, PartitionSpec("core"))
        dev_in = [jax.device_put(a, sh) for a in concat_in]
        for a in dev_in:
            a.block_until_ready()
        times = []
        for _ in range(time_runs):
            dz = [jax.device_put(z, sh) for z in concat_zeros]
            for z in dz:
                z.block_until_ready()
            t0 = time.perf_counter()
            outs = sharded(*dev_in, *dz)
            for o in outs:
                o.block_until_ready()
            times.append(time.perf_counter() - t0)
        exec_ns = int(min(times) * 1e9)
    return results, exec_ns


# --------------------------------------------------------------------------
# Entry point
# --------------------------------------------------------------------------

def kernel(feat, src, dst, W1, b1, W2, b2):
    global LAST_EXEC_NS
    import ml_dtypes

    feat = np.asarray(feat, dtype=np.float32)
    src = np.asarray(src, dtype=np.int64)
    dst = np.asarray(dst, dtype=np.int64)
    W1 = np.asarray(W1, dtype=np.float32)
    W2 = np.asarray(W2, dtype=np.float32)
    b1 = np.asarray(b1, dtype=np.float32)
    b2 = np.asarray(b2, dtype=np.float32)
    N = feat.shape[0]

    t0 = time.perf_counter()
    S = _preprocess(src, dst, N, NCORES, STAB, RG)
    Q, T = S["Q"], S["T"]
    has_b1 = bool(np.any(b1))
    has_b2 = bool(np.any(b2))
    t1 = time.perf_counter()
    print(f"[kernel] preprocess {t1 - t0:.1f}s", file=sys.stderr)

    key = ("nc", N, Q, T, has_b1, has_b2)
    if key not in _CACHE:
        _CACHE[key] = _build_nc(S, N, STAB, has_b1, has_b2)
    nc = _CACHE[key]
    t2 = time.perf_counter()
    print(f"[kernel] build+schedule {t2 - t1:.1f}s", file=sys.stderr)

    bf = ml_dtypes.bfloat16
    tab1 = feat.astype(bf)
    r128 = np.tile(np.arange(128, dtype=np.float32), (128, 1)).astype(bf)
    id128 = np.eye(128, dtype=np.float32)

    in_maps = []
    for c in range(NCORES):
        m = {
            "table1": tab1,
            "ndcol": S["ndcol"][c],
            "r128": r128,
            "id128": id128,
            "w0": W1,
            "w1": W2,
        }
        for q in range(Q):
            m[f"idx{q}"] = S["idx_sb"][c][q]
            m[f"dof{q}"] = S["dstoff_sb"][c][q]
            m[f"nsr{q}"] = S["nsrc_sb"][c][q]
        if has_b1:
            m["b0"] = b1.reshape(1, D)
        if has_b2:
            m["b1"] = b2.reshape(1, D)
        in_maps.append(m)

    t3 = time.perf_counter()
    results, exec_ns = _run_and_time(nc, in_maps, NCORES)
    t4 = time.perf_counter()
    print(f"[kernel] compile+run+time {t4 - t3:.1f}s exec_ns={exec_ns}",
          file=sys.stderr)
    LAST_EXEC_NS = exec_ns
    out = np.concatenate([results[c]["out"] for c in range(NCORES)], axis=0)
    return out[:N].astype(np.float32)
